# revision 16
# baseline (speedup 1.0000x reference)
"""Trainium2 Bass kernel for nn_AttentionBlock (pre-LN causal attention + SiLU MLP).

8-core SPMD strategy (data-parallel over batch x sequence-parallel over rows):
  - core c handles sample b = c // NPOS, position g = c % NPOS
  - the L rows of a sample are split into NBLK blocks of BS rows; each core owns
    NBPC blocks, paired to balance causal-attention cost (host-chosen pairing)
  - every core computes LN1 + K^T/V for the full sample (replicated), q/proj/MLP
    only for its own rows.  Per-core differences in causal trip counts are
    handled with tc.If branches on partition_id; per-core data differences
    (mask_len row selection) are handled via input data (sel blend / vbar).

All matmul layouts are "transposed" (feature dim on partitions) so no on-device
transposes are needed anywhere; the host feeds x pre-transposed and re-assembles
the transposed output.
"""
import math
from contextlib import ExitStack
from dataclasses import dataclass

import ml_dtypes
import numpy as np

import concourse.bass as bass
import concourse.mybir as mybir
import concourse.tile as tile
from concourse import bacc
from concourse.bass import ds, ts
from concourse.bass_utils import run_bass_kernel_spmd

F32 = mybir.dt.float32
BF16 = mybir.dt.bfloat16
AF = mybir.ActivationFunctionType
ALU = mybir.AluOpType
BF16NP = ml_dtypes.bfloat16


@dataclass
class Cfg:
    B: int = 2
    L: int = 2048
    E: int = 768
    H: int = 12
    D: int = 64
    FF: int = 3072
    BS: int = 256          # query block rows
    n_cores: int = 8
    eps: float = 1e-6

    @property
    def NPOS(self):
        return self.n_cores // self.B

    @property
    def NBLK(self):
        return self.L // self.BS

    @property
    def NBPC(self):
        return self.NBLK // self.NPOS   # blocks per core

    @property
    def R(self):
        return self.NBPC * self.BS      # own rows per core

    @property
    def EC(self):
        return self.E // 128

    @property
    def FC(self):
        return self.FF // 128

    @property
    def LC(self):
        return self.L // 128

    @property
    def HC(self):
        return self.H // 2              # head-pair chunks (= EC since E = H*D, D=64)


def plan_blocks(cfg: Cfg, mask_lens):
    """Choose jmax (number of attention-active blocks) and block pairing."""
    mmax = int(max(int(m) for m in mask_lens))
    mmax = max(1, min(cfg.L, mmax))
    jmax = (mmax + cfg.BS - 1) // cfg.BS          # blocks [0, jmax) need causal attn
    def cost(j):
        return (j + 1) if j < jmax else 0
    order = sorted(range(cfg.NBLK), key=lambda j: -cost(j))
    pairs = []
    for g in range(cfg.NPOS):
        blocks = []
        for s in range(cfg.NBPC):
            # snake over sorted order: pair heavy with light
            idx = g if s % 2 == 0 else (cfg.NBLK - 1 - g)
            blocks.append(order[idx])
        pairs.append(tuple(blocks))
    return pairs, jmax


def kc_of(cfg: Cfg, j, jmax):
    """number of 128-wide key chunks block j attends to (0 if mask-free)."""
    if j >= jmax:
        return 0
    return (j + 1) * cfg.BS // 128


# ----------------------------------------------------------------------------
# program builder
# ----------------------------------------------------------------------------

def build_program(cfg: Cfg, pairs, jmax, flags, bake_g=None, stage_limit=99):
    """flags: dict with bools: bq, bk, bv, bproj, bfc, bout, ln1aff, ln2aff

    bake_g: if set, emit only that variant's attention without tc.If (for
    timing estimation with TimelineSim)."""
    E, L, H, D2, FF, BS, R = cfg.E, cfg.L, cfg.H, cfg.D, cfg.FF, cfg.BS, cfg.R
    EC, FC, LC, HC, NBPC = cfg.EC, cfg.FC, cfg.LC, cfg.HC, cfg.NBPC
    KEYS = jmax * BS
    KC = KEYS // 128
    qscale = 1.0 / math.sqrt(cfg.D)

    nc = bacc.Bacc(num_devices=cfg.n_cores)

    # ---- dram I/O ----
    d_xTf = nc.dram_tensor("xT_full", [E, L], BF16, kind="ExternalInput")
    d_xTo = nc.dram_tensor("xT_own", [E, R], F32, kind="ExternalInput")
    d_wq = nc.dram_tensor("wq", [EC, 128, E], BF16, kind="ExternalInput")
    d_wk = nc.dram_tensor("wk", [EC, 128, E], BF16, kind="ExternalInput")
    d_wv = nc.dram_tensor("wv", [EC, 128, E], BF16, kind="ExternalInput")
    d_wp = nc.dram_tensor("wproj", [EC, 128, E], BF16, kind="ExternalInput")
    d_wfc = nc.dram_tensor("wfc", [FC, EC, 128, 128], BF16, kind="ExternalInput")
    d_wout = nc.dram_tensor("wout", [EC, FC, 128, 128], BF16, kind="ExternalInput")
    d_bq = nc.dram_tensor("bq", [128, EC], F32, kind="ExternalInput")
    d_bk = nc.dram_tensor("bk", [128, EC], F32, kind="ExternalInput")
    d_bv = nc.dram_tensor("bv", [1, E], BF16, kind="ExternalInput")
    d_bp = nc.dram_tensor("bproj", [128, EC], F32, kind="ExternalInput")
    d_bfc = nc.dram_tensor("bfc", [128, FC], F32, kind="ExternalInput")
    d_bout = nc.dram_tensor("bout", [128, EC], F32, kind="ExternalInput")
    d_ln = nc.dram_tensor("lnp", [4, 128, EC], F32, kind="ExternalInput")  # g1,b1,g2,b2
    d_selb = nc.dram_tensor("selb", [128, R], BF16, kind="ExternalInput")
    d_masks = nc.dram_tensor("diagmasks", [2, 128, BS], BF16, kind="ExternalInput")
    d_out = nc.dram_tensor("outT", [E, R], F32, kind="ExternalOutput")

    with tile.TileContext(nc) as tc, ExitStack() as st:
        # ------- L0: persistent pools -------
        cpool = st.enter_context(tc.tile_pool(name="consts", bufs=1))
        gpsum = st.enter_context(tc.tile_pool(name="gpsum", bufs=2, space="PSUM"))

        wq_s = cpool.tile([128, EC, E], BF16)
        nc.sync.dma_start(wq_s[:], d_wq.rearrange("c p n -> p c n"))
        wk_s = cpool.tile([128, EC, E], BF16)
        nc.sync.dma_start(wk_s[:], d_wk.rearrange("c p n -> p c n"))
        wv_s = cpool.tile([128, EC, E], BF16)
        nc.sync.dma_start(wv_s[:], d_wv.rearrange("c p n -> p c n"))
        wp_s = cpool.tile([128, EC, E], BF16)
        nc.sync.dma_start(wp_s[:], d_wp.rearrange("c p n -> p c n"))
        xo_s = cpool.tile([128, EC, R], F32)
        nc.sync.dma_start(xo_s[:], d_xTo.rearrange("(c p) n -> p c n", p=128))
        bq_s = cpool.tile([128, EC], F32)
        nc.sync.dma_start(bq_s[:], d_bq[:])
        bk_s = cpool.tile([128, EC], F32)
        nc.sync.dma_start(bk_s[:], d_bk[:])
        bv_s = cpool.tile([1, E], BF16)
        nc.sync.dma_start(bv_s[:], d_bv[:])
        bp_s = cpool.tile([128, EC], F32)
        nc.sync.dma_start(bp_s[:], d_bp[:])
        bfc_s = cpool.tile([128, FC], F32)
        nc.sync.dma_start(bfc_s[:], d_bfc[:])
        bout_s = cpool.tile([128, EC], F32)
        nc.sync.dma_start(bout_s[:], d_bout[:])
        ln_s = cpool.tile([128, 4, EC], F32)
        nc.sync.dma_start(ln_s[:], d_ln.rearrange("k p c -> p k c"))
        selb_s = cpool.tile([128, R], BF16)
        nc.sync.dma_start(selb_s[:], d_selb[:])
        maskA = cpool.tile([128, BS], BF16)
        nc.sync.dma_start(maskA[:], d_masks[0])
        maskB = cpool.tile([128, BS], BF16)
        nc.sync.dma_start(maskB[:], d_masks[1])

        ones_col = cpool.tile([128, 1], BF16)
        nc.vector.memset(ones_col[:], 1.0)
        ones_row = cpool.tile([1, 128], BF16)
        nc.vector.memset(ones_row[:], 1.0)
        ones_11 = cpool.tile([1, 1], BF16)
        nc.vector.memset(ones_11[:], 1.0)
        eps_11 = cpool.tile([1, 1], F32)
        nc.vector.memset(eps_11[:], cfg.eps)

        yT = cpool.tile([128, HC, R], BF16)
        nc.vector.memset(yT[:], 0.0)
        vbarT = cpool.tile([128, EC, 1], F32)
        vrow = cpool.tile([1, E], BF16)

        # ============================================================
        # helper: layernorm in transposed layout
        # x_bf: sbuf [128, EC, N] bf16 ; writes zT [128, EC, N] bf16
        # ============================================================
        def ln_transposed(pool, x_bf, N, gb_idx, zT_out, tag):
            """zT_out may alias x_bf (in-place LN apply)."""
            gi, bi = gb_idx
            affine = flags["ln1aff"] if gb_idx == (0, 1) else flags["ln2aff"]
            for cg0 in range(0, N, 512):
                w = min(512, N - cg0)
                ps_su = gpsum.tile([1, 512], F32, tag="gp", name=f"pssu{tag}{cg0}")
                ps_sq = gpsum.tile([1, 512], F32, tag="gp", name=f"pssq{tag}{cg0}")
                for c in range(EC):
                    nc.tensor.matmul(ps_su[:, :w], ones_col[:], x_bf[:, c, cg0:cg0 + w],
                                     start=(c == 0), stop=(c == EC - 1))
                for c in range(EC):
                    sq = pool.tile([128, 512], BF16, tag="lnsq", name=f"sq{tag}{cg0}{c}")
                    nc.vector.tensor_tensor(sq[:, :w], x_bf[:, c, cg0:cg0 + w],
                                            x_bf[:, c, cg0:cg0 + w], ALU.mult)
                    nc.tensor.matmul(ps_sq[:, :w], ones_col[:], sq[:, :w],
                                     start=(c == 0), stop=(c == EC - 1))
                # mu = sum/E ; var = sumsq/E - mu^2 ; a = 1/sqrt(var+eps) ; b = -mu*a
                mu = pool.tile([1, 512], F32, tag="lnmu", name=f"mu{tag}{cg0}")
                nc.vector.tensor_scalar_mul(mu[:, :w], ps_su[:, :w], 1.0 / E)
                va = pool.tile([1, 512], F32, tag="lnva", name=f"va{tag}{cg0}")
                nc.vector.tensor_scalar_mul(va[:, :w], ps_sq[:, :w], 1.0 / E)
                t1 = pool.tile([1, 512], F32, tag="lnt1", name=f"t1{tag}{cg0}")
                nc.vector.tensor_tensor(t1[:, :w], mu[:, :w], mu[:, :w], ALU.mult)
                nc.vector.tensor_sub(va[:, :w], va[:, :w], t1[:, :w])
                nc.scalar.activation(t1[:, :w], va[:, :w], AF.Sqrt, bias=eps_11[:])
                nc.vector.reciprocal(va[:, :w], t1[:, :w])     # va = rstd = a
                arow = pool.tile([1, 512], BF16, tag="lnar", name=f"ar{tag}{cg0}")
                nc.vector.tensor_copy(arow[:, :w], va[:, :w])
                nc.vector.tensor_tensor(t1[:, :w], mu[:, :w], va[:, :w], ALU.mult)
                brow = pool.tile([1, 512], BF16, tag="lnbr", name=f"br{tag}{cg0}")
                nc.vector.tensor_scalar_mul(brow[:, :w], t1[:, :w], -1.0)
                ab = pool.tile([128, 2, 512], BF16, tag="lnab", name=f"ab{tag}{cg0}")
                nc.gpsimd.partition_broadcast(ab[:, 0, :w], arow[:, :w], channels=128)
                nc.gpsimd.partition_broadcast(ab[:, 1, :w], brow[:, :w], channels=128)
                for c in range(EC):
                    nc.vector.tensor_tensor(zT_out[:, c, cg0:cg0 + w],
                                            x_bf[:, c, cg0:cg0 + w], ab[:, 0, :w], ALU.mult)
                    nc.vector.tensor_tensor(zT_out[:, c, cg0:cg0 + w],
                                            zT_out[:, c, cg0:cg0 + w], ab[:, 1, :w], ALU.add)
                    if affine:
                        nc.vector.tensor_scalar(zT_out[:, c, cg0:cg0 + w],
                                                zT_out[:, c, cg0:cg0 + w],
                                                ln_s[:, gi, c:c + 1], ln_s[:, bi, c:c + 1],
                                                ALU.mult, ALU.add)

        # ------- L2: sample-wide tensors (die after attention) -------
        with tc.tile_pool(name="l2", bufs=1) as l2:
            # zT / zqT are computed in place over the loaded x tiles
            zT = l2.tile([128, EC, L], BF16, tag="zT", name="zT")
            nc.sync.dma_start(zT[:], d_xTf.rearrange("(c p) n -> p c n", p=128))
            zqT = l2.tile([128, EC, R], BF16, tag="zqT", name="zqT")
            nc.vector.tensor_copy(zqT[:], xo_s[:])
            qTs = l2.tile([128, HC, R], BF16, tag="qTs", name="qTs")
            kTs = l2.tile([128, HC, KEYS], BF16, tag="kTs", name="kTs")
            Vs = l2.tile([128, LC, H, 65], BF16, tag="Vs", name="Vs")

            # ------- L3: LN1 scratch (dies after zT/zqT written) -------
            if stage_limit >= 1:
                with tc.tile_pool(name="l3", bufs=2) as l3:
                    ln_transposed(l3, zT, L, (0, 1), zT, "f")
                    ln_transposed(l3, zqT, R, (0, 1), zqT, "o")

            # ------- QKV -------
            # q^T (own rows): [128(hd), HC, R]
            for m in range(EC if stage_limit >= 2 else 0):
                ps = gpsum.tile([128, 512], F32, tag="gp", name=f"psq{m}")
                for c in range(EC):
                    nc.tensor.matmul(ps[:, :R], wq_s[:, c, ts(m, 128)], zqT[:, c, :],
                                     start=(c == 0), stop=(c == EC - 1))
                if flags["bq"]:
                    nc.vector.tensor_scalar(qTs[:, m, :], ps[:, :R], bq_s[:, m:m + 1],
                                            qscale, ALU.add, ALU.mult)
                else:
                    nc.vector.tensor_scalar_mul(qTs[:, m, :], ps[:, :R], qscale)
            # k^T (keys 0..KEYS): [128(hd), HC, KEYS]
            for m in range(EC if stage_limit >= 2 else 0):
                for n0 in range(0, KEYS, 512):
                    w = min(512, KEYS - n0)
                    ps = gpsum.tile([128, 512], F32, tag="gp", name=f"psk{m}{n0}")
                    for c in range(EC):
                        nc.tensor.matmul(ps[:, :w], wk_s[:, c, ts(m, 128)],
                                         zT[:, c, n0:n0 + w],
                                         start=(c == 0), stop=(c == EC - 1))
                    if flags["bk"]:
                        nc.vector.tensor_scalar(kTs[:, m, n0:n0 + w], ps[:, :w],
                                                bk_s[:, m:m + 1], None, ALU.add)
                    else:
                        nc.vector.tensor_copy(kTs[:, m, n0:n0 + w], ps[:, :w])
            # V natural: [128(keyrow), LC, H, 0:64], col 64 = 1.0
            nc.vector.memset(Vs[:, :, :, 64:65], 1.0)
            for r in range(LC if stage_limit >= 2 else 0):
                for n0 in range(0, E, 512):
                    w = min(512, E - n0)
                    ps = gpsum.tile([128, 512], F32, tag="gp", name=f"psv{r}{n0}")
                    for c in range(EC):
                        nc.tensor.matmul(ps[:, :w], zT[:, c, ts(r, 128)],
                                         wv_s[:, c, n0:n0 + w],
                                         start=(c == 0),
                                         stop=(c == EC - 1 and not flags["bv"]))
                    if flags["bv"]:
                        nc.tensor.matmul(ps[:, :w], ones_row[:], bv_s[:, n0:n0 + w],
                                         start=False, stop=True)
                    h0 = n0 // 64
                    nh = w // 64
                    nc.vector.tensor_copy(
                        Vs[:, r, h0:h0 + nh, 0:64],
                        ps[:, :w].rearrange("p (h d) -> p h d", d=64))

            # ------- attention -------
            with (
                tc.tile_pool(name="att", bufs=3) as att,
                tc.tile_pool(name="spsum", bufs=2, space="PSUM") as spsum,
                tc.tile_pool(name="ypsum", bufs=2, space="PSUM") as ypsum,
            ):
                from contextlib import nullcontext
                gvar = None if bake_g is not None else nc.partition_id() % cfg.NPOS
                for g in range(cfg.NPOS if stage_limit >= 3 else 0):
                    if bake_g is not None and g != bake_g:
                        continue
                    with (nullcontext() if bake_g is not None else tc.If(gvar == g)):
                        for slot in range(NBPC):
                            j = pairs[g][slot]
                            kc = kc_of(cfg, j, jmax)
                            if kc == 0:
                                continue
                            qsl = ds(slot * BS, BS)
                            for hp in range(HC):
                                ps_ys = []
                                for h01 in (0, 1):
                                    ps_y = ypsum.tile([65, BS], F32, tag="y",
                                                      name=f"y{g}{slot}{hp}{h01}")
                                    ps_ys.append(ps_y)
                                kdone = 0
                                while kdone < kc:
                                    gsz = min(4, kc - kdone)
                                    for h01 in (0, 1):
                                        h = 2 * hp + h01
                                        pb = h01 * 64
                                        ps_s = spsum.tile([128, 4, BS], F32, tag="s",
                                                          name=f"s{g}{slot}{hp}{h01}{kdone}")
                                        for i in range(gsz):
                                            ki = kdone + i
                                            nc.tensor.matmul(
                                                ps_s[:, i, :],
                                                kTs[pb:pb + 64, hp, ts(ki, 128)],
                                                qTs[pb:pb + 64, hp, qsl],
                                                start=True, stop=True)
                                        ex = att.tile([128, 4, BS], BF16, tag="ex",
                                                      name=f"ex{g}{slot}{hp}{h01}{kdone}")
                                        nc.scalar.activation(ex[:, :gsz, :], ps_s[:, :gsz, :], AF.Exp)
                                        for i in range(gsz):
                                            ki = kdone + i
                                            if ki == kc - 2:
                                                nc.vector.tensor_tensor(ex[:, i, :], ex[:, i, :], maskA[:], ALU.mult)
                                            elif ki == kc - 1:
                                                nc.vector.tensor_tensor(ex[:, i, :], ex[:, i, :], maskB[:], ALU.mult)
                                        for i in range(gsz):
                                            ki = kdone + i
                                            nc.tensor.matmul(
                                                ps_ys[h01][:],
                                                Vs[:, ki, h, :],
                                                ex[:, i, :],
                                                start=(ki == 0), stop=(ki == kc - 1))
                                    kdone += gsz
                                for h01 in (0, 1):
                                    pb = h01 * 64
                                    rr = att.tile([1, BS], F32, tag="rr",
                                                  name=f"rr{g}{slot}{hp}{h01}")
                                    nc.vector.reciprocal(rr[:], ps_ys[h01][64:65, :])
                                    rb = att.tile([64, BS], F32, tag="rb",
                                                  name=f"rb{g}{slot}{hp}{h01}")
                                    nc.gpsimd.partition_broadcast(rb[:], rr[:], channels=64)
                                    nc.vector.tensor_tensor(yT[pb:pb + 64, hp, qsl],
                                                            ps_ys[h01][0:64, :], rb[:], ALU.mult)

            # vbar = mean over all L rows of V, per head -> vbarT [128, EC, 1]
            HG = 512 // 65            # heads per vbar psum group
            for h0 in range(0, H if stage_limit >= 4 else 0, HG):
                nh = min(HG, H - h0)
                ps = gpsum.tile([1, 512], F32, tag="gp", name=f"vb{h0}")
                for r in range(LC):
                    nc.tensor.matmul(ps[:, :nh * 65], ones_col[:],
                                     Vs[:, r, h0:h0 + nh, :],
                                     start=(r == 0), stop=(r == LC - 1))
                nc.vector.tensor_scalar_mul(
                    vrow[:, h0 * 64:(h0 + nh) * 64].rearrange("p (h d) -> p h d", d=64),
                    ps[:, :nh * 65].rearrange("p (h c) -> p h c", c=65)[:, :, 0:64],
                    1.0 / L)
            for m in range(EC if stage_limit >= 4 else 0):
                ps = gpsum.tile([128, 512], F32, tag="gp", name=f"vbt{m}")
                nc.tensor.matmul(ps[:, 0:1], vrow[:, ts(m, 128)], ones_11[:],
                                 start=True, stop=True)
                nc.vector.tensor_copy(vbarT[:, m, :], ps[:, 0:1])

            # blend: yT = vbar + (yT - vbar) * sel
            vb_b = vbarT[:].to_broadcast([128, EC, R])
            sel_b = selb_s[:, None, :].to_broadcast([128, EC, R])
            nc.vector.tensor_tensor(yT[:], yT[:], vb_b, ALU.subtract)
            nc.vector.tensor_tensor(yT[:], yT[:], sel_b, ALU.mult)
            nc.vector.tensor_tensor(yT[:], yT[:], vb_b, ALU.add)

        # ------- L2c: proj / LN2 / MLP -------
        with tc.tile_pool(name="l2c", bufs=1) as l2c:
            x1T = l2c.tile([128, EC, R], F32)
            x1b = l2c.tile([128, EC, R], BF16)
            z2T = l2c.tile([128, EC, R], BF16)
            hT = l2c.tile([128, FC, R], BF16)
            outT = l2c.tile([128, EC, R], F32)

            for m in range(EC if stage_limit >= 5 else 0):
                ps = gpsum.tile([128, 512], F32, tag="gp", name=f"psp{m}")
                for c in range(HC):
                    nc.tensor.matmul(ps[:, :R], wp_s[:, c, ts(m, 128)], yT[:, c, :],
                                     start=(c == 0), stop=(c == HC - 1))
                nc.vector.tensor_tensor(x1T[:, m, :], ps[:, :R], xo_s[:, m, :], ALU.add)
                if flags["bproj"]:
                    nc.vector.tensor_scalar(x1T[:, m, :], x1T[:, m, :],
                                            bp_s[:, m:m + 1], None, ALU.add)
                nc.vector.tensor_copy(x1b[:, m, :], x1T[:, m, :])

            if stage_limit >= 5:
                with tc.tile_pool(name="l3c", bufs=1) as l3c:
                    ln_transposed(l3c, x1b, R, (2, 3), z2T, "2")

            with tc.tile_pool(name="wstream", bufs=2) as wstream:
                for m in range(FC if stage_limit >= 6 else 0):
                    wfc_m = wstream.tile([128, EC, 128], BF16, tag="wfc", name=f"wfc{m}")
                    nc.sync.dma_start(wfc_m[:], d_wfc[m].rearrange("c p n -> p c n"))
                    ps = gpsum.tile([128, 512], F32, tag="gp", name=f"psh{m}")
                    for c in range(EC):
                        nc.tensor.matmul(ps[:, :R], wfc_m[:, c, :], z2T[:, c, :],
                                         start=(c == 0), stop=(c == EC - 1))
                    sg = wstream.tile([128, R], BF16, tag="sg", name=f"sg{m}")
                    if flags["bfc"]:
                        nc.scalar.activation(sg[:], ps[:, :R], AF.Sigmoid,
                                             bias=bfc_s[:, m:m + 1])
                        t2 = wstream.tile([128, R], F32, tag="t2", name=f"t2{m}")
                        nc.vector.tensor_scalar(t2[:], ps[:, :R],
                                                bfc_s[:, m:m + 1], None, ALU.add)
                        nc.vector.tensor_tensor(hT[:, m, :], t2[:], sg[:], ALU.mult)
                    else:
                        nc.scalar.activation(sg[:], ps[:, :R], AF.Sigmoid)
                        nc.vector.tensor_tensor(hT[:, m, :], ps[:, :R], sg[:], ALU.mult)
                for m in range(EC if stage_limit >= 6 else 0):
                    wout_m = wstream.tile([128, FC, 128], BF16, tag="wout", name=f"wout{m}")
                    nc.sync.dma_start(wout_m[:], d_wout[m].rearrange("k p n -> p k n"))
                    ps = gpsum.tile([128, 512], F32, tag="gp", name=f"pso{m}")
                    for k in range(FC):
                        nc.tensor.matmul(ps[:, :R], wout_m[:, k, :], hT[:, k, :],
                                         start=(k == 0), stop=(k == FC - 1))
                    nc.vector.tensor_tensor(outT[:, m, :], ps[:, :R], x1T[:, m, :], ALU.add)
                    if flags["bout"]:
                        nc.vector.tensor_scalar(outT[:, m, :], outT[:, m, :],
                                                bout_s[:, m:m + 1], None, ALU.add)

            nc.sync.dma_start(d_out.rearrange("(c p) n -> p c n", p=128), outT[:])

    nc.finalize()
    return nc


# ----------------------------------------------------------------------------
# host side: input prep / output assembly
# ----------------------------------------------------------------------------

def prepare_in_maps(cfg: Cfg, pairs, jmax, flags, inputs):
    """Build per-core input maps. Returns (in_maps, percore_blocks)."""
    x = np.asarray(inputs["x"], np.float32)
    w_qkv = np.asarray(inputs["w_qkv"], np.float32)
    b_qkv = np.asarray(inputs["b_qkv"], np.float32)
    w_proj = np.asarray(inputs["w_proj"], np.float32)
    b_proj = np.asarray(inputs["b_proj"], np.float32)
    w_fc = np.asarray(inputs["w_fc"], np.float32)
    b_fc = np.asarray(inputs["b_fc"], np.float32)
    w_out = np.asarray(inputs["w_out"], np.float32)
    b_out = np.asarray(inputs["b_out"], np.float32)
    ln1_s = np.asarray(inputs["ln1_scale"], np.float32)
    ln1_b = np.asarray(inputs["ln1_bias"], np.float32)
    ln2_s = np.asarray(inputs["ln2_scale"], np.float32)
    ln2_b = np.asarray(inputs["ln2_bias"], np.float32)
    mask_len = np.asarray(inputs["mask_len"]).astype(np.int64)

    E, L, H, D, BS = cfg.E, cfg.L, cfg.H, cfg.D, cfg.BS
    EC, FC = cfg.EC, cfg.FC

    # split qkv columns: col = h*3D + {0..D-1:q, D..2D-1:k, 2D..3D-1:v}
    wsplit = w_qkv.reshape(E, H, 3 * D)
    wq = np.ascontiguousarray(wsplit[:, :, 0:D].reshape(E, E))
    wk = np.ascontiguousarray(wsplit[:, :, D:2 * D].reshape(E, E))
    wv = np.ascontiguousarray(wsplit[:, :, 2 * D:3 * D].reshape(E, E))
    bsplit = b_qkv.reshape(H, 3 * D)
    bq = np.ascontiguousarray(bsplit[:, 0:D].reshape(E))
    bk = np.ascontiguousarray(bsplit[:, D:2 * D].reshape(E))
    bv = np.ascontiguousarray(bsplit[:, 2 * D:3 * D].reshape(E))

    def chunked_w(w):  # [E, N] -> [EC, 128, N] bf16
        return np.ascontiguousarray(w.reshape(EC, 128, -1)).astype(BF16NP)

    def col_f32(v):    # [E or FF] -> [128, C]
        return np.ascontiguousarray(v.reshape(-1, 128).T).astype(np.float32)

    wq_c, wk_c, wv_c, wp_c = (chunked_w(w) for w in (wq, wk, wv, w_proj))
    wfc_c = np.ascontiguousarray(
        w_fc.reshape(EC, 128, FC, 128).transpose(2, 0, 1, 3)).astype(BF16NP)
    wout_c = np.ascontiguousarray(
        w_out.reshape(FC, 128, EC, 128).transpose(2, 0, 1, 3)).astype(BF16NP)
    lnp = np.stack([col_f32(ln1_s), col_f32(ln1_b), col_f32(ln2_s), col_f32(ln2_b)])

    ki = np.arange(128)[:, None]
    qi = np.arange(BS)[None, :]
    masks = np.stack([(qi >= ki), (qi >= ki + 128)]).astype(BF16NP)

    shared = dict(
        wq=wq_c, wk=wk_c, wv=wv_c, wproj=wp_c, wfc=wfc_c, wout=wout_c,
        bq=col_f32(bq), bk=col_f32(bk), bv=bv.reshape(1, E).astype(BF16NP),
        bproj=col_f32(b_proj), bfc=col_f32(b_fc), bout=col_f32(b_out),
        lnp=lnp, diagmasks=masks,
    )

    in_maps = []
    percore_blocks = []
    for c in range(cfg.n_cores):
        b = c // cfg.NPOS
        g = c % cfg.NPOS
        blocks = pairs[g]
        percore_blocks.append((b, blocks))
        xT = x[b].T  # [E, L]
        own_cols = np.concatenate(
            [np.arange(j * BS, (j + 1) * BS) for j in blocks])
        sel = (own_cols < mask_len[b]).astype(BF16NP)
        selb = np.broadcast_to(sel[None, :], (128, cfg.R))
        m = dict(shared)
        m["xT_full"] = np.ascontiguousarray(xT).astype(BF16NP)
        m["xT_own"] = np.ascontiguousarray(xT[:, own_cols]).astype(np.float32)
        m["selb"] = np.ascontiguousarray(selb)
        in_maps.append(m)
    return in_maps, percore_blocks


def assemble_output(cfg: Cfg, results, percore_blocks):
    out = np.zeros((cfg.B, cfg.L, cfg.E), np.float32)
    for c, res in enumerate(results):
        b, blocks = percore_blocks[c]
        oT = res["outT"]  # [E, R]
        for s, j in enumerate(blocks):
            out[b, j * cfg.BS:(j + 1) * cfg.BS, :] = oT[:, s * cfg.BS:(s + 1) * cfg.BS].T
    return out


def make_flags(inputs):
    def nz(name):
        return bool(np.any(np.asarray(inputs[name]) != 0))
    return dict(
        bq=nz("b_qkv"), bk=nz("b_qkv"), bv=nz("b_qkv"),
        bproj=nz("b_proj"), bfc=nz("b_fc"), bout=nz("b_out"),
        ln1aff=bool(np.any(np.asarray(inputs["ln1_scale"]) != 1)
                    or np.any(np.asarray(inputs["ln1_bias"]) != 0)),
        ln2aff=bool(np.any(np.asarray(inputs["ln2_scale"]) != 1)
                    or np.any(np.asarray(inputs["ln2_bias"]) != 0)),
    )


_cached = {}


def kernel(**inputs) -> np.ndarray:
    cfg = Cfg()
    mask_len = np.asarray(inputs["mask_len"]).astype(np.int64)
    pairs, jmax = plan_blocks(cfg, mask_len)
    flags = make_flags(inputs)
    key = (tuple(map(tuple, pairs)), jmax, tuple(sorted(flags.items())))
    if key not in _cached:
        _cached[key] = build_program(cfg, pairs, jmax, flags)
    nc = _cached[key]
    in_maps, percore_blocks = prepare_in_maps(cfg, pairs, jmax, flags, inputs)
    r = run_bass_kernel_spmd(nc, in_maps, core_ids=list(range(cfg.n_cores)))
    return assemble_output(cfg, r.results, percore_blocks)


if __name__ == "__main__":
    pass
